# revision 72
# baseline (speedup 1.0000x reference)
"""Causal GQA attention (B=4, S=2048, H=16, HK=4, D=128) on 8 trn2 cores.

Sharding: 16 (request, kv-head) units, 2 per core. Each unit owns 4 query
heads that share one K/V head (GQA group).

Per-core kernel (per head):
  - scores are computed TRANSPOSED: S_T[sk, sq] = K_chunk^T-stationary
    matmul with Q^T moving. Q/K are bf16 (host-converted): 1 cyc/row at any
    free size (f32r runs 4 cyc/row on the 128-wide diagonal chunks) and
    half the input DMA bytes. No P-transposes are ever needed.
  - exp on ScalarE straight out of PSUM with the 1/sqrt(D) scale folded in,
    output bf16 to SBUF. ScalarE is the bottleneck engine (~136us busy);
    everything else is scheduled around keeping it fed.
  - causal masking is multiplicative on the two diagonal chunks per group.
  - PV: lhsT = expT chunk [sk,128sq] (bf16 stationary), rhs = V' chunk
    [sk, 129] where column 128 is ones => row-sums accumulate for free in
    PSUM column 128, and the output lands already [sq, d].
  - PV matmuls are queued as single-matmul pieces and drained <=KPV at a
    time between score batches: PE is in-order, so small injected PV blocks
    never park the next QK batch (and its exp) behind a long PV run.
  - normalize with DVE reciprocal + tensor_scalar_mul, DMA out.
  - DMA transfers serialize in ready-order in the cost model: k (3 pieces)
    + q + v ride SP in consumption order, the tiny opening-q pieces ride
    the Pool SWDGE queue, and the ACT HWDGE queue is kept clear for exps.
  - tail: the last head's final batch completes groups 0 then 1 (ascending)
    with per-group stores, tail PV tiles rotate through the idle scores
    banks, and half the tail normalizes run as ACT Copy(scale=1/rowsum).
"""

import sys

if "/opt/trn_rl_repo" not in sys.path:
    sys.path.insert(0, "/opt/trn_rl_repo")

from contextlib import ExitStack

import ml_dtypes
import numpy as np

import concourse.bass as bass  # noqa: F401  (registers engine classes)
import concourse.tile as tile
from concourse import bacc, mybir
from concourse.bass_utils import run_bass_kernel_spmd

B = 4
S = 2048
H = 16
HK = 4
D = 128
REP = H // HK          # query heads per kv head
SCALE = float(1.0 / np.float32(np.sqrt(D)))

NCORES = 8
NU = 2                 # units (request, kv-head) per core
NHL = REP              # query heads per unit
P = 128
NT = S // P            # 16 sk/sq tiles per sequence
SQG = 256              # sq columns per group (>=256 keeps f32r at full rate)
NG = S // SQG          # 8 groups per head
GB = 6                 # score chunks per PSUM batch ([128, 6*256] = 3 banks)

F32 = mybir.dt.float32
F32R = mybir.dt.float32r
BF16 = mybir.dt.bfloat16

_CACHE = {}


def _build_program(qk_f32r: bool = False, pipe: int = 3, split_loads: bool = False):
    nc = bacc.Bacc("TRN2", target_bir_lowering=False, debug=False,
                   num_devices=NCORES)
    QKDT = BF16
    qT = nc.dram_tensor("qT", [NU, NHL, P, S], QKDT, kind="ExternalInput").ap()
    kT = nc.dram_tensor("kT", [NU, P, S], QKDT, kind="ExternalInput").ap()
    vp = nc.dram_tensor("vp", [NU, P, NT, D + 1], BF16, kind="ExternalInput").ap()
    masks = nc.dram_tensor("masks", [P, P], BF16, kind="ExternalInput").ap()
    # [unit, head, sq_partition, q_tile, d] -- one DMA per head, 8KB rows
    out = nc.dram_tensor("out", [NU, NHL, P, NT, D], F32, kind="ExternalOutput").ap()

    with tile.TileContext(nc) as tc, ExitStack() as ctx:
        kpool = ctx.enter_context(tc.tile_pool(name="kT", bufs=2))
        qpool = ctx.enter_context(tc.tile_pool(name="qT", bufs=2))
        vpool = ctx.enter_context(tc.tile_pool(name="vp", bufs=2))
        epool = ctx.enter_context(tc.tile_pool(name="expT", bufs=3))
        mpool = ctx.enter_context(tc.tile_pool(name="masks", bufs=1))
        opool = ctx.enter_context(tc.tile_pool(name="osb", bufs=2))
        rpool = ctx.enter_context(tc.tile_pool(name="recip", bufs=4))
        spsum = ctx.enter_context(tc.tile_pool(name="scores", bufs=2, space="PSUM"))
        ppsum = ctx.enter_context(tc.tile_pool(name="pv", bufs=2, space="PSUM"))

        mtile = mpool.tile([P, P], BF16)

        # PE HAM warm-up: dummy matmuls on scratch SBUF during the initial
        # K/Q DMA wait so the first real QK batch runs above the low pstate.
        # Sized to END roughly when the first K/Q pieces land (~2.1us).
        wpool = ctx.enter_context(tc.tile_pool(name="warm", bufs=1))
        wtile = wpool.tile([P, 512], BF16)
        nc.vector.memset(wtile[:].bitcast(mybir.dt.uint16), 0)
        for _ in range(2):
            wpsum = ppsum.tile([P, D + 1], F32, tag="pv", name="wpsum")
            nc.tensor.matmul(wpsum[:], lhsT=wtile[:, :P],
                             rhs=wtile[:, :D + 1], start=True, stop=True)

        HEADCOLS = sum((2 * g + 1) * SQG + P for g in range(NG))  # 17408
        BCOLS = GB * SQG                 # psum batch capacity (3 banks)

        # Software pipeline: PV work is queued as individual matmul pieces
        # and drained a few at a time between QK/exp batches. PE is in-order,
        # so keeping each injected PV block small (KPV matmuls) guarantees
        # the next QK batch -- and with it ScalarE's next exp -- is never
        # parked behind a long PV run.
        pvwork = []        # list of zero-arg closures, each emits 1 PE op
        KPV = 10           # max PV matmuls injected per batch boundary

        def queue_pv(vtile, etile, colof, osb, g, store, pool, act_norm=False):
            """Queue PV + normalize pieces for the two q-tiles of group g."""
            state = {}

            def start_tile(cc):
                t = 2 * g + cc
                if pool is None or cc == 1:
                    ptile = ppsum.tile([P, D + 1], F32, tag="pv", name="ptile")
                else:
                    # tail PVs rotate through the (by then idle) scores
                    # banks so the last four PV tiles land in four distinct
                    # banks and never wait on a normalize read
                    ptile = pool.tile([P, BCOLS], F32, tag="scores",
                                      name="tailpv")[:, :D + 1]
                state[cc] = ptile
                return ptile, t

            def mm(cc, j):
                def run():
                    if j == 0:
                        ptile, t = start_tile(cc)
                    else:
                        ptile, t = state[cc], 2 * g + cc
                    c0 = colof[(g, j)] + (0 if (cc == 1 and j == t) else cc * P)
                    nc.tensor.matmul(
                        ptile[:],
                        lhsT=etile[:, c0:c0 + P],
                        rhs=vtile[:, j, :],
                        start=(j == 0), stop=(j == t),
                    )
                    if j == t:
                        rec = rpool.tile([P, 1], F32)
                        nc.vector.reciprocal(rec[:], ptile[:, D:D + 1])
                        if act_norm and cc == 1:
                            # tail tiles: run half the normalizes on the (by
                            # then idle) ACT engine so the final normalize ->
                            # store chain isn't serialized on DVE
                            nc.scalar.activation(
                                osb[:, t, :], ptile[:, 0:D],
                                mybir.ActivationFunctionType.Copy,
                                scale=rec[:])
                        else:
                            nc.vector.tensor_scalar_mul(osb[:, t, :],
                                                        ptile[:, 0:D], rec[:])
                        # store: list fires after cc==1; dict fires per-cc
                        if isinstance(store, dict):
                            for eng, dst, src in store.get(cc, ()):
                                eng.dma_start(dst, src)
                        elif cc == 1 and store is not None:
                            for eng, dst, src in store:
                                eng.dma_start(dst, src)
                return run

            for cc in range(2):
                for j in range(2 * g + cc + 1):
                    pvwork.append(mm(cc, j))

        def drain_pv(n):
            for _ in range(min(n, len(pvwork))):
                pvwork.pop(0)()

        for u in range(NU):
            ktile = kpool.tile([P, S], QKDT)
            if u == 0:
                # The cost model serializes DMA transfers in ready-order, so
                # split k into 3 pieces on SP (consumption order) and put the
                # tiny opening-q pieces + masks on the Pool SWDGE queue; the
                # ACT queue stays clear for exp dispatch (its LoadActFuncSet
                # blocks early DMA generation there).
                nc.sync.dma_start(ktile[:, :384], kT[u][:, :384])
                nc.sync.dma_start(ktile[:, 384:1152], kT[u][:, 384:1152])
                nc.sync.dma_start(ktile[:, 1152:], kT[u][:, 1152:])
            else:
                nc.sync.dma_start(ktile[:], kT[u])
            vtile = vpool.tile([P, NT, D + 1], BF16)
            for hl in range(NHL):
                qtile = qpool.tile([P, S], QKDT)
                if u == 0 and hl == 0:
                    # groups run big->small and the first batch is 3 chunks:
                    # the opening batches only read q columns [1792:2048]
                    nc.gpsimd.dma_start(qtile[:, 1792:], qT[u, hl][:, 1792:])
                    nc.gpsimd.dma_start(qtile[:, 1536:1792],
                                        qT[u, hl][:, 1536:1792])
                    nc.sync.dma_start(vtile[:], vp[u])
                    nc.sync.dma_start(qtile[:, :1536], qT[u, hl][:, :1536])
                    nc.gpsimd.dma_start(mtile[:], masks[:])
                elif hl == 0:
                    nc.sync.dma_start(qtile[:], qT[u, hl])
                    nc.sync.dma_start(vtile[:], vp[u])
                else:
                    nc.sync.dma_start(qtile[:], qT[u, hl])
                osb = opool.tile([P, NT, D], F32)
                etile = epool.tile([P, HEADCOLS], BF16)
                last_head = (u == NU - 1 and hl == NHL - 1)
                # groups big->small: every head ends on tiny PV work and the
                # next head opens with big QK batches, keeping ScalarE fed
                # across head boundaries (and the kernel tail short)
                gs = list(range(NG - 1, -1, -1))

                # chunk stream for this head: full 256-wide chunks of a PAIR
                # of groups, then their two 128-wide diagonal half-chunks
                # back-to-back. Keeps every 256-wide PSUM write 256-aligned so
                # no matmul output crosses a PSUM bank boundary.
                chunks = []
                for ga, gb in zip(gs[0::2], gs[1::2]):
                    for j in range(2 * ga + 1):
                        chunks.append((ga, j, SQG, ga * SQG))
                    chunks.append((ga, 2 * ga + 1, P, ga * SQG + P))
                    chunks.append((gb, 2 * gb + 1, P, gb * SQG + P))
                    for j in range(2 * gb + 1):
                        chunks.append((gb, j, SQG, gb * SQG))
                remaining = {g: 2 * g + 2 for g in gs}
                colof = {}
                acc = 0
                for (g, j, w, qc) in chunks:
                    colof[(g, j)] = acc
                    acc += w

                def do_batch(batch, bcols):
                    stile = spsum.tile([P, BCOLS], F32, tag="scores")
                    ncols = 0
                    for (g, j, w, qc) in batch:
                        nc.tensor.matmul(
                            stile[:, ncols:ncols + w],
                            lhsT=ktile[:, j * P:(j + 1) * P],
                            rhs=qtile[:, qc:qc + w],
                            start=True, stop=True,
                        )
                        ncols += w
                    e0 = colof[(batch[0][0], batch[0][1])]
                    nc.scalar.activation(
                        etile[:, e0:e0 + ncols],
                        stile[:, :ncols],
                        mybir.ActivationFunctionType.Exp,
                        scale=SCALE,
                    )
                    # a group is complete once all its chunks are exp'd.
                    # Groups complete big->small (descending) which keeps the
                    # steady-state pipeline tight; only the LAST head's final
                    # batch (groups 1+0 complete together) queues ascending,
                    # so g0's normalize+store starts ~1us earlier at the tail.
                    for (g, j, w, qc) in batch:
                        remaining[g] -= 1
                    done = [g for g in dict.fromkeys(c[0] for c in batch)
                            if remaining[g] == 0]
                    if last_head and set(done) == {0, 1}:
                        # only the very last batch flips to ascending: stores
                        # are per-group there, so g0's chain can lead. Other
                        # multi-group batches must stay descending -- e.g.
                        # g2's store reads g3's osb tiles and therefore has
                        # to queue after g3's normalizes.
                        done = sorted(done)
                    for g in done:
                        if True:
                            for tcol in (colof[(g, 2 * g)],
                                         colof[(g, 2 * g + 1)]):
                                nc.vector.tensor_mul(
                                    etile[:, tcol:tcol + P],
                                    etile[:, tcol:tcol + P], mtile[:])
                            if last_head and g == NG // 2:
                                # tiles 8..15 done early under big->small
                                # order: store them as soon as ready
                                store = [(nc.gpsimd, out[u, hl][:, NG:, :],
                                          osb[:, NG:, :])]
                            elif last_head and g == 2:
                                store = [(nc.sync, out[u, hl][:, 4:6, :],
                                          osb[:, 4:6, :]),
                                         (nc.gpsimd, out[u, hl][:, 6:NG, :],
                                          osb[:, 6:NG, :])]
                            elif last_head and g == 1:
                                # g1 is queued after g0 (ascending), so this
                                # is the final store of the kernel
                                store = [(nc.sync, out[u, hl][:, 2:4, :],
                                          osb[:, 2:4, :])]
                            elif last_head and g == 0:
                                store = [(nc.scalar, out[u, hl][:, 0:2, :],
                                          osb[:, 0:2, :])]
                            elif g == gs[-1]:
                                # descending order: g0 is queued last, so the
                                # whole-head store (which reads every osb
                                # tile) rides on its final normalize
                                store = [(nc.gpsimd, out[u, hl], osb[:])]
                            else:
                                store = None
                            # the last head's final two PV groups run after
                            # the last exp: pull their PSUM from the (by then
                            # idle) scores pool so they don't serialize on the
                            # two pv banks behind group 2's normalize.
                            pvpool = spsum if (last_head and g <= 1) else None
                            queue_pv(vtile, etile, colof, osb, g, store,
                                     pvpool,
                                     act_norm=(last_head and g <= 3))

                batch, bcols = [], 0
                nbatch = 0
                for ch in chunks:
                    # the first batch of each head is kept small (3 chunks) so
                    # the next head's opening QK+exp slots in right behind the
                    # previous head's final exp instead of stalling ScalarE.
                    # On the last head the final batch is split at the g1/g0
                    # boundary so group 1 completes (and its PV+normalize
                    # chain starts) while group 0's exp still runs.
                    cap = ((512 if u == 0 and hl == 0 else 1024)
                           if nbatch == 0 else BCOLS)
                    split = (last_head and ch[0] == 0 and batch
                             and batch[-1][0] == 1)
                    if bcols + ch[2] > cap or split:
                        do_batch(batch, bcols)
                        nbatch += 1
                        batch, bcols = [], 0
                        # on the last head drain harder so only the final two
                        # groups' PV work remains after the last exp -- but
                        # defer entirely on the last two boundaries so the
                        # tail masks lead the in-order DVE stream
                        if last_head and nbatch >= 11:
                            pass
                        else:
                            drain_pv(KPV + (8 if last_head else 0))
                    batch.append(ch)
                    bcols += ch[2]
                if batch:
                    do_batch(batch, bcols)
        drain_pv(len(pvwork))

    nc.compile()
    return nc


def _prep_inputs(q, k, v):
    """Host-side sharding/layout. Returns in_maps for the 8 cores."""
    q = np.ascontiguousarray(np.asarray(q, dtype=np.float32))
    k = np.ascontiguousarray(np.asarray(k, dtype=np.float32))
    v = np.ascontiguousarray(np.asarray(v, dtype=np.float32))

    # [B, H(K), D, S] transposed views, contiguous
    qt_all = np.ascontiguousarray(q.reshape(B, S, H, D).transpose(0, 2, 3, 1))
    kt_all = np.ascontiguousarray(k.reshape(B, S, HK, D).transpose(0, 2, 3, 1))
    v4 = v.reshape(B, S, HK, D)

    # tri mask for scoresT diagonal blocks: [r, c] = 1 if c >= r
    r = np.arange(P)[:, None]
    c = np.arange(P)[None, :]
    masks = (c >= r).astype(ml_dtypes.bfloat16)

    in_maps = []
    for core in range(NCORES):
        qs = np.empty((NU, NHL, P, S), dtype=ml_dtypes.bfloat16)
        ks = np.empty((NU, P, S), dtype=ml_dtypes.bfloat16)
        vs = np.empty((NU, P, NT, D + 1), dtype=ml_dtypes.bfloat16)
        for ui in range(NU):
            ug = core * NU + ui
            b, kv = divmod(ug, HK)
            qs[ui] = qt_all[b, kv * REP:(kv + 1) * REP]
            ks[ui] = kt_all[b, kv]
            # V' chunks: [sk_in_chunk, chunk, d] with ones in column D
            vu = v4[b, :, kv, :].reshape(NT, P, D).transpose(1, 0, 2)
            vs[ui, :, :, :D] = vu.astype(ml_dtypes.bfloat16)
            vs[ui, :, :, D] = 1.0
        in_maps.append({"qT": qs, "kT": ks, "vp": vs, "masks": masks})
    return in_maps


def _assemble(results):
    res = np.empty((B, S, H, D), dtype=np.float32)
    for core in range(NCORES):
        arr = results[core]["out"]  # [NU, NHL, P(sq), NT, D]
        for ui in range(NU):
            ug = core * NU + ui
            b, kv = divmod(ug, HK)
            # [NHL, P, NT, D] -> [(NT P) = S, NHL, D]
            blk = arr[ui].transpose(2, 1, 0, 3).reshape(S, NHL, D)
            res[b, :, kv * REP:(kv + 1) * REP, :] = blk
    return res.reshape(B * S, H * D)


def kernel(q, k, v, seq_lens=None, **_unused):
    key = "prog"
    if key not in _CACHE:
        _CACHE[key] = _build_program()
    nc = _CACHE[key]
    in_maps = _prep_inputs(q, k, v)
    res = run_bass_kernel_spmd(nc, in_maps, list(range(NCORES)))
    return _assemble(res.results)



# revision 75
# speedup vs baseline: 1.0032x; 1.0032x over previous
"""Causal GQA attention (B=4, S=2048, H=16, HK=4, D=128) on 8 trn2 cores.

Sharding: 16 (request, kv-head) units, 2 per core. Each unit owns 4 query
heads that share one K/V head (GQA group).

Per-core kernel (per head):
  - scores are computed TRANSPOSED: S_T[sk, sq] = K_chunk^T-stationary
    matmul with Q^T moving. Q/K are bf16 (host-converted): 1 cyc/row at any
    free size (f32r runs 4 cyc/row on the 128-wide diagonal chunks) and
    half the input DMA bytes. No P-transposes are ever needed.
  - exp on ScalarE straight out of PSUM with the 1/sqrt(D) scale folded in,
    output bf16 to SBUF. ScalarE is the bottleneck engine (~136us busy);
    everything else is scheduled around keeping it fed.
  - causal masking is multiplicative on the two diagonal chunks per group.
  - PV: lhsT = expT chunk [sk,128sq] (bf16 stationary), rhs = V' chunk
    [sk, 129] where column 128 is ones => row-sums accumulate for free in
    PSUM column 128, and the output lands already [sq, d].
  - PV matmuls are queued as single-matmul pieces and drained <=KPV at a
    time between score batches: PE is in-order, so small injected PV blocks
    never park the next QK batch (and its exp) behind a long PV run.
  - normalize with DVE reciprocal + tensor_scalar_mul, DMA out.
  - DMA transfers serialize in ready-order in the cost model: k (3 pieces)
    + q + v ride SP in consumption order, the tiny opening-q pieces ride
    the Pool SWDGE queue, and the ACT HWDGE queue is kept clear for exps.
  - tail: the last head's final batch completes groups 0 then 1 (ascending)
    with per-group stores, tail PV tiles rotate through the idle scores
    banks, and half the tail normalizes run as ACT Copy(scale=1/rowsum).
"""

import sys

if "/opt/trn_rl_repo" not in sys.path:
    sys.path.insert(0, "/opt/trn_rl_repo")

from contextlib import ExitStack

import ml_dtypes
import numpy as np

import concourse.bass as bass  # noqa: F401  (registers engine classes)
import concourse.tile as tile
from concourse import bacc, mybir
from concourse.bass_utils import run_bass_kernel_spmd

B = 4
S = 2048
H = 16
HK = 4
D = 128
REP = H // HK          # query heads per kv head
SCALE = float(1.0 / np.float32(np.sqrt(D)))

NCORES = 8
NU = 2                 # units (request, kv-head) per core
NHL = REP              # query heads per unit
P = 128
NT = S // P            # 16 sk/sq tiles per sequence
SQG = 256              # sq columns per group (>=256 keeps f32r at full rate)
NG = S // SQG          # 8 groups per head
GB = 6                 # score chunks per PSUM batch ([128, 6*256] = 3 banks)

F32 = mybir.dt.float32
F32R = mybir.dt.float32r
BF16 = mybir.dt.bfloat16

_CACHE = {}


def _build_program(qk_f32r: bool = False, pipe: int = 3, split_loads: bool = False):
    nc = bacc.Bacc("TRN2", target_bir_lowering=False, debug=False,
                   num_devices=NCORES)
    QKDT = BF16
    qT = nc.dram_tensor("qT", [NU, NHL, P, S], QKDT, kind="ExternalInput").ap()
    kT = nc.dram_tensor("kT", [NU, P, S], QKDT, kind="ExternalInput").ap()
    vp = nc.dram_tensor("vp", [NU, P, NT, D + 1], BF16, kind="ExternalInput").ap()
    masks = nc.dram_tensor("masks", [P, 2, P], BF16, kind="ExternalInput").ap()
    # [unit, head, sq_partition, q_tile, d] -- one DMA per head, 8KB rows
    out = nc.dram_tensor("out", [NU, NHL, P, NT, D], F32, kind="ExternalOutput").ap()

    with tile.TileContext(nc) as tc, ExitStack() as ctx:
        kpool = ctx.enter_context(tc.tile_pool(name="kT", bufs=2))
        qpool = ctx.enter_context(tc.tile_pool(name="qT", bufs=2))
        vpool = ctx.enter_context(tc.tile_pool(name="vp", bufs=2))
        epool = ctx.enter_context(tc.tile_pool(name="expT", bufs=3))
        mpool = ctx.enter_context(tc.tile_pool(name="masks", bufs=1))
        opool = ctx.enter_context(tc.tile_pool(name="osb", bufs=2))
        rpool = ctx.enter_context(tc.tile_pool(name="recip", bufs=4))
        spsum = ctx.enter_context(tc.tile_pool(name="scores", bufs=2, space="PSUM"))
        ppsum = ctx.enter_context(tc.tile_pool(name="pv", bufs=2, space="PSUM"))

        mtile = mpool.tile([P, 2, P], BF16)

        # PE HAM warm-up: dummy matmuls on scratch SBUF during the initial
        # K/Q DMA wait so the first real QK batch runs above the low pstate.
        # Sized to END roughly when the first K/Q pieces land (~2.1us).
        wpool = ctx.enter_context(tc.tile_pool(name="warm", bufs=1))
        wtile = wpool.tile([P, 512], BF16)
        nc.vector.memset(wtile[:].bitcast(mybir.dt.uint16), 0)
        for _ in range(2):
            wpsum = ppsum.tile([P, D + 1], F32, tag="pv", name="wpsum")
            nc.tensor.matmul(wpsum[:], lhsT=wtile[:, :P],
                             rhs=wtile[:, :D + 1], start=True, stop=True)

        HEADCOLS = sum((2 * g + 1) * SQG + P for g in range(NG))  # 17408
        BCOLS = GB * SQG                 # psum batch capacity (3 banks)

        # Software pipeline: PV work is queued as individual matmul pieces
        # and drained a few at a time between QK/exp batches. PE is in-order,
        # so keeping each injected PV block small (KPV matmuls) guarantees
        # the next QK batch -- and with it ScalarE's next exp -- is never
        # parked behind a long PV run.
        pvwork = []        # list of zero-arg closures, each emits 1 PE op
        KPV = 10           # max PV matmuls injected per batch boundary

        def queue_pv(vtile, etile, colof, osb, g, store, pool, act_norm=False):
            """Queue PV + normalize pieces for the two q-tiles of group g."""
            state = {}

            def start_tile(cc):
                t = 2 * g + cc
                if pool is None or cc == 1:
                    ptile = ppsum.tile([P, D + 1], F32, tag="pv", name="ptile")
                else:
                    # tail PVs rotate through the (by then idle) scores
                    # banks so the last four PV tiles land in four distinct
                    # banks and never wait on a normalize read
                    ptile = pool.tile([P, BCOLS], F32, tag="scores",
                                      name="tailpv")[:, :D + 1]
                state[cc] = ptile
                return ptile, t

            def mm(cc, j):
                def run():
                    if j == 0:
                        ptile, t = start_tile(cc)
                    else:
                        ptile, t = state[cc], 2 * g + cc
                    c0 = colof[(g, j)] + (0 if (cc == 1 and j == t) else cc * P)
                    nc.tensor.matmul(
                        ptile[:],
                        lhsT=etile[:, c0:c0 + P],
                        rhs=vtile[:, j, :],
                        start=(j == 0), stop=(j == t),
                    )
                    if j == t:
                        rec = rpool.tile([P, 1], F32)
                        nc.vector.reciprocal(rec[:], ptile[:, D:D + 1])
                        if act_norm and cc == 1:
                            # tail tiles: run half the normalizes on the (by
                            # then idle) ACT engine so the final normalize ->
                            # store chain isn't serialized on DVE
                            nc.scalar.activation(
                                osb[:, t, :], ptile[:, 0:D],
                                mybir.ActivationFunctionType.Copy,
                                scale=rec[:])
                        else:
                            nc.vector.tensor_scalar_mul(osb[:, t, :],
                                                        ptile[:, 0:D], rec[:])
                        # store: list fires after cc==1; dict fires per-cc
                        if isinstance(store, dict):
                            for eng, dst, src in store.get(cc, ()):
                                eng.dma_start(dst, src)
                        elif cc == 1 and store is not None:
                            for eng, dst, src in store:
                                eng.dma_start(dst, src)
                return run

            for cc in range(2):
                for j in range(2 * g + cc + 1):
                    pvwork.append(mm(cc, j))

        def drain_pv(n):
            for _ in range(min(n, len(pvwork))):
                pvwork.pop(0)()

        for u in range(NU):
            ktile = kpool.tile([P, S], QKDT)
            if u == 0:
                # The cost model serializes DMA transfers in ready-order, so
                # split k into 3 pieces on SP (consumption order) and put the
                # tiny opening-q pieces + masks on the Pool SWDGE queue; the
                # ACT queue stays clear for exp dispatch (its LoadActFuncSet
                # blocks early DMA generation there).
                nc.sync.dma_start(ktile[:, :384], kT[u][:, :384])
                nc.sync.dma_start(ktile[:, 384:1152], kT[u][:, 384:1152])
                nc.sync.dma_start(ktile[:, 1152:], kT[u][:, 1152:])
            else:
                nc.sync.dma_start(ktile[:], kT[u])
            vtile = vpool.tile([P, NT, D + 1], BF16)
            for hl in range(NHL):
                qtile = qpool.tile([P, S], QKDT)
                if u == 0 and hl == 0:
                    # groups run big->small and the first batch is 3 chunks:
                    # the opening batches only read q columns [1792:2048]
                    nc.gpsimd.dma_start(qtile[:, 1792:], qT[u, hl][:, 1792:])
                    nc.gpsimd.dma_start(mtile[:], masks[:])
                    nc.gpsimd.dma_start(qtile[:, 1536:1792],
                                        qT[u, hl][:, 1536:1792])
                    nc.sync.dma_start(vtile[:], vp[u])
                    nc.sync.dma_start(qtile[:, :1536], qT[u, hl][:, :1536])
                elif hl == 0:
                    nc.sync.dma_start(qtile[:], qT[u, hl])
                    nc.sync.dma_start(vtile[:], vp[u])
                else:
                    nc.sync.dma_start(qtile[:], qT[u, hl])
                osb = opool.tile([P, NT, D], F32)
                etile = epool.tile([P, HEADCOLS], BF16)
                last_head = (u == NU - 1 and hl == NHL - 1)
                # groups big->small: every head ends on tiny PV work and the
                # next head opens with big QK batches, keeping ScalarE fed
                # across head boundaries (and the kernel tail short)
                gs = list(range(NG - 1, -1, -1))

                # chunk stream for this head: full 256-wide chunks of a PAIR
                # of groups, then their two 128-wide diagonal half-chunks
                # back-to-back. Keeps every 256-wide PSUM write 256-aligned so
                # no matmul output crosses a PSUM bank boundary.
                chunks = []
                for ga, gb in zip(gs[0::2], gs[1::2]):
                    for j in range(2 * ga + 1):
                        chunks.append((ga, j, SQG, ga * SQG))
                    chunks.append((ga, 2 * ga + 1, P, ga * SQG + P))
                    chunks.append((gb, 2 * gb + 1, P, gb * SQG + P))
                    for j in range(2 * gb + 1):
                        chunks.append((gb, j, SQG, gb * SQG))
                remaining = {g: 2 * g + 2 for g in gs}
                colof = {}
                acc = 0
                for (g, j, w, qc) in chunks:
                    colof[(g, j)] = acc
                    acc += w

                def do_batch(batch, bcols):
                    stile = spsum.tile([P, BCOLS], F32, tag="scores")
                    ncols = 0
                    for (g, j, w, qc) in batch:
                        # diagonal chunks get the causal mask folded in on PE:
                        # a second matmul accumulates -1e9 into the masked
                        # triangle (exp then gives exactly 0), so no DVE mask
                        # multiply is ever needed downstream
                        diag = (j == 2 * g or j == 2 * g + 1)
                        nc.tensor.matmul(
                            stile[:, ncols:ncols + w],
                            lhsT=ktile[:, j * P:(j + 1) * P],
                            rhs=qtile[:, qc:qc + w],
                            start=True, stop=not diag,
                        )
                        if diag:
                            nc.tensor.matmul(
                                stile[:, ncols:ncols + P],
                                lhsT=mtile[:, 0, :],
                                rhs=mtile[:, 1, :],
                                start=False, stop=True,
                            )
                        ncols += w
                    e0 = colof[(batch[0][0], batch[0][1])]
                    nc.scalar.activation(
                        etile[:, e0:e0 + ncols],
                        stile[:, :ncols],
                        mybir.ActivationFunctionType.Exp,
                        scale=SCALE,
                    )
                    # a group is complete once all its chunks are exp'd.
                    # Groups complete big->small (descending) which keeps the
                    # steady-state pipeline tight; only the LAST head's final
                    # batch (groups 1+0 complete together) queues ascending,
                    # so g0's normalize+store starts ~1us earlier at the tail.
                    for (g, j, w, qc) in batch:
                        remaining[g] -= 1
                    done = [g for g in dict.fromkeys(c[0] for c in batch)
                            if remaining[g] == 0]
                    if last_head and set(done) == {0, 1}:
                        # only the very last batch flips to ascending: stores
                        # are per-group there, so g0's chain can lead. Other
                        # multi-group batches must stay descending -- e.g.
                        # g2's store reads g3's osb tiles and therefore has
                        # to queue after g3's normalizes.
                        done = sorted(done)
                    for g in done:
                        if True:
                            if last_head and g == NG // 2:
                                # tiles 8..15 done early under big->small
                                # order: store them as soon as ready
                                store = [(nc.gpsimd, out[u, hl][:, NG:, :],
                                          osb[:, NG:, :])]
                            elif last_head and g == 2:
                                store = [(nc.sync, out[u, hl][:, 4:6, :],
                                          osb[:, 4:6, :]),
                                         (nc.gpsimd, out[u, hl][:, 6:NG, :],
                                          osb[:, 6:NG, :])]
                            elif last_head and g == 1:
                                # g1 is queued after g0 (ascending), so this
                                # is the final store of the kernel
                                store = [(nc.sync, out[u, hl][:, 2:4, :],
                                          osb[:, 2:4, :])]
                            elif last_head and g == 0:
                                store = [(nc.scalar, out[u, hl][:, 0:2, :],
                                          osb[:, 0:2, :])]
                            elif g == gs[-1]:
                                # descending order: g0 is queued last, so the
                                # whole-head store (which reads every osb
                                # tile) rides on its final normalize
                                store = [(nc.gpsimd, out[u, hl], osb[:])]
                            else:
                                store = None
                            # the last head's final two PV groups run after
                            # the last exp: pull their PSUM from the (by then
                            # idle) scores pool so they don't serialize on the
                            # two pv banks behind group 2's normalize.
                            pvpool = spsum if (last_head and g <= 1) else None
                            queue_pv(vtile, etile, colof, osb, g, store,
                                     pvpool,
                                     act_norm=(last_head and g <= 3))

                batch, bcols = [], 0
                nbatch = 0
                for ch in chunks:
                    # the first batch of each head is kept small (3 chunks) so
                    # the next head's opening QK+exp slots in right behind the
                    # previous head's final exp instead of stalling ScalarE.
                    # On the last head the final batch is split at the g1/g0
                    # boundary so group 1 completes (and its PV+normalize
                    # chain starts) while group 0's exp still runs.
                    cap = ((512 if u == 0 and hl == 0 else 1024)
                           if nbatch == 0 else BCOLS)
                    split = (last_head and ch[0] == 0 and batch
                             and batch[-1][0] == 1)
                    if bcols + ch[2] > cap or split:
                        do_batch(batch, bcols)
                        nbatch += 1
                        batch, bcols = [], 0
                        # on the last head drain harder so only the final two
                        # groups' PV work remains after the last exp -- but
                        # defer entirely on the last two boundaries so the
                        # tail masks lead the in-order DVE stream
                        if last_head and nbatch >= 11:
                            pass
                        else:
                            drain_pv(KPV + (8 if last_head else 0))
                    batch.append(ch)
                    bcols += ch[2]
                if batch:
                    do_batch(batch, bcols)
        drain_pv(len(pvwork))

    nc.compile()
    return nc


def _prep_inputs(q, k, v):
    """Host-side sharding/layout. Returns in_maps for the 8 cores."""
    q = np.ascontiguousarray(np.asarray(q, dtype=np.float32))
    k = np.ascontiguousarray(np.asarray(k, dtype=np.float32))
    v = np.ascontiguousarray(np.asarray(v, dtype=np.float32))

    # [B, H(K), D, S] transposed views, contiguous
    qt_all = np.ascontiguousarray(q.reshape(B, S, H, D).transpose(0, 2, 3, 1))
    kt_all = np.ascontiguousarray(k.reshape(B, S, HK, D).transpose(0, 2, 3, 1))
    v4 = v.reshape(B, S, HK, D)

    # PE-side causal mask for scoresT diagonal blocks: masks[:,0,:] is
    # (-1e9 * [sk > sq])^T, masks[:,1,:] the identity; lhsT.T @ rhs adds
    # -1e9 to masked positions (exp -> 0) and exactly 0 elsewhere.
    r = np.arange(P)[:, None]
    c = np.arange(P)[None, :]
    masks = np.empty((P, 2, P), dtype=ml_dtypes.bfloat16)
    masks[:, 0, :] = np.where(c > r, -1e9, 0.0).astype(ml_dtypes.bfloat16)
    masks[:, 1, :] = np.eye(P, dtype=np.float32)

    in_maps = []
    for core in range(NCORES):
        qs = np.empty((NU, NHL, P, S), dtype=ml_dtypes.bfloat16)
        ks = np.empty((NU, P, S), dtype=ml_dtypes.bfloat16)
        vs = np.empty((NU, P, NT, D + 1), dtype=ml_dtypes.bfloat16)
        for ui in range(NU):
            ug = core * NU + ui
            b, kv = divmod(ug, HK)
            qs[ui] = qt_all[b, kv * REP:(kv + 1) * REP]
            ks[ui] = kt_all[b, kv]
            # V' chunks: [sk_in_chunk, chunk, d] with ones in column D
            vu = v4[b, :, kv, :].reshape(NT, P, D).transpose(1, 0, 2)
            vs[ui, :, :, :D] = vu.astype(ml_dtypes.bfloat16)
            vs[ui, :, :, D] = 1.0
        in_maps.append({"qT": qs, "kT": ks, "vp": vs, "masks": masks})
    return in_maps


def _assemble(results):
    res = np.empty((B, S, H, D), dtype=np.float32)
    for core in range(NCORES):
        arr = results[core]["out"]  # [NU, NHL, P(sq), NT, D]
        for ui in range(NU):
            ug = core * NU + ui
            b, kv = divmod(ug, HK)
            # [NHL, P, NT, D] -> [(NT P) = S, NHL, D]
            blk = arr[ui].transpose(2, 1, 0, 3).reshape(S, NHL, D)
            res[b, :, kv * REP:(kv + 1) * REP, :] = blk
    return res.reshape(B * S, H * D)


def kernel(q, k, v, seq_lens=None, **_unused):
    key = "prog"
    if key not in _CACHE:
        _CACHE[key] = _build_program()
    nc = _CACHE[key]
    in_maps = _prep_inputs(q, k, v)
    res = run_bass_kernel_spmd(nc, in_maps, list(range(NCORES)))
    return _assemble(res.results)



# revision 77
# speedup vs baseline: 1.0036x; 1.0004x over previous
"""Causal GQA attention (B=4, S=2048, H=16, HK=4, D=128) on 8 trn2 cores.

Sharding: 16 (request, kv-head) units, 2 per core. Each unit owns 4 query
heads that share one K/V head (GQA group).

Per-core kernel (per head):
  - scores are computed TRANSPOSED: S_T[sk, sq] = K_chunk^T-stationary
    matmul with Q^T moving. Q/K are bf16 (host-converted): 1 cyc/row at any
    free size (f32r runs 4 cyc/row on the 128-wide diagonal chunks) and
    half the input DMA bytes. No P-transposes are ever needed.
  - exp on ScalarE straight out of PSUM with the 1/sqrt(D) scale folded in,
    output bf16 to SBUF. ScalarE is the bottleneck engine (~136us busy);
    everything else is scheduled around keeping it fed.
  - causal masking is folded into the QK matmul on PE: one extra
    accumulate adds -1e9 to each diagonal chunk's masked triangle
    (exp -> exactly 0), so DVE never touches a mask.
  - PV: lhsT = expT chunk [sk,128sq] (bf16 stationary), rhs = V' chunk
    [sk, 129] where column 128 is ones => row-sums accumulate for free in
    PSUM column 128, and the output lands already [sq, d].
  - PV matmuls are queued as single-matmul pieces and drained <=KPV at a
    time between score batches: PE is in-order, so small injected PV blocks
    never park the next QK batch (and its exp) behind a long PV run.
  - normalize with DVE reciprocal + tensor_scalar_mul, DMA out.
  - DMA transfers serialize in ready-order in the cost model: k (3 pieces)
    + q + v ride SP in consumption order, the tiny opening-q pieces ride
    the Pool SWDGE queue, and the ACT HWDGE queue is kept clear for exps.
  - tail: the last head's final batch completes groups 0 then 1 (ascending)
    with per-group stores, tail PV tiles rotate through the idle scores
    banks, and half the tail normalizes run as ACT Copy(scale=1/rowsum).
"""

import sys

if "/opt/trn_rl_repo" not in sys.path:
    sys.path.insert(0, "/opt/trn_rl_repo")

from contextlib import ExitStack

import ml_dtypes
import numpy as np

import concourse.bass as bass  # noqa: F401  (registers engine classes)
import concourse.tile as tile
from concourse import bacc, mybir
from concourse.bass_utils import run_bass_kernel_spmd

B = 4
S = 2048
H = 16
HK = 4
D = 128
REP = H // HK          # query heads per kv head
SCALE = float(1.0 / np.float32(np.sqrt(D)))

NCORES = 8
NU = 2                 # units (request, kv-head) per core
NHL = REP              # query heads per unit
P = 128
NT = S // P            # 16 sk/sq tiles per sequence
SQG = 256              # sq columns per group (>=256 keeps f32r at full rate)
NG = S // SQG          # 8 groups per head
GB = 6                 # score chunks per PSUM batch ([128, 6*256] = 3 banks)

F32 = mybir.dt.float32
F32R = mybir.dt.float32r
BF16 = mybir.dt.bfloat16

_CACHE = {}


def _build_program(qk_f32r: bool = False, pipe: int = 3, split_loads: bool = False):
    nc = bacc.Bacc("TRN2", target_bir_lowering=False, debug=False,
                   num_devices=NCORES)
    QKDT = BF16
    qT = nc.dram_tensor("qT", [NU, NHL, P, S], QKDT, kind="ExternalInput").ap()
    kT = nc.dram_tensor("kT", [NU, P, S], QKDT, kind="ExternalInput").ap()
    vp = nc.dram_tensor("vp", [NU, P, NT, D + 1], BF16, kind="ExternalInput").ap()
    masks = nc.dram_tensor("masks", [P, 2, P], BF16, kind="ExternalInput").ap()
    # [unit, head, sq_partition, q_tile, d] -- one DMA per head, 8KB rows
    out = nc.dram_tensor("out", [NU, NHL, P, NT, D], F32, kind="ExternalOutput").ap()

    with tile.TileContext(nc) as tc, ExitStack() as ctx:
        kpool = ctx.enter_context(tc.tile_pool(name="kT", bufs=2))
        qpool = ctx.enter_context(tc.tile_pool(name="qT", bufs=2))
        vpool = ctx.enter_context(tc.tile_pool(name="vp", bufs=2))
        epool = ctx.enter_context(tc.tile_pool(name="expT", bufs=3))
        mpool = ctx.enter_context(tc.tile_pool(name="masks", bufs=1))
        opool = ctx.enter_context(tc.tile_pool(name="osb", bufs=2))
        rpool = ctx.enter_context(tc.tile_pool(name="recip", bufs=4))
        spsum = ctx.enter_context(tc.tile_pool(name="scores", bufs=2, space="PSUM"))
        ppsum = ctx.enter_context(tc.tile_pool(name="pv", bufs=2, space="PSUM"))

        mtile = mpool.tile([P, 2, P], BF16)

        # PE HAM warm-up: dummy matmuls on scratch SBUF during the initial
        # K/Q DMA wait so the first real QK batch runs above the low pstate.
        # Sized to END roughly when the first K/Q pieces land (~2.1us).
        wpool = ctx.enter_context(tc.tile_pool(name="warm", bufs=1))
        wtile = wpool.tile([P, 512], BF16)
        nc.vector.memset(wtile[:].bitcast(mybir.dt.uint16), 0)
        for _ in range(2):
            wpsum = ppsum.tile([P, D + 1], F32, tag="pv", name="wpsum")
            nc.tensor.matmul(wpsum[:], lhsT=wtile[:, :P],
                             rhs=wtile[:, :D + 1], start=True, stop=True)

        HEADCOLS = sum((2 * g + 1) * SQG + P for g in range(NG))  # 17408
        BCOLS = GB * SQG                 # psum batch capacity (3 banks)

        # Software pipeline: PV work is queued as individual matmul pieces
        # and drained a few at a time between QK/exp batches. PE is in-order,
        # so keeping each injected PV block small (KPV matmuls) guarantees
        # the next QK batch -- and with it ScalarE's next exp -- is never
        # parked behind a long PV run.
        pvwork = []        # list of zero-arg closures, each emits 1 PE op
        KPV = 10           # max PV matmuls injected per batch boundary

        def queue_pv(vtile, etile, colof, osb, g, store, pool, act_norm=False):
            """Queue PV + normalize pieces for the two q-tiles of group g."""
            state = {}

            def start_tile(cc):
                t = 2 * g + cc
                if pool is None or cc == 1:
                    ptile = ppsum.tile([P, D + 1], F32, tag="pv", name="ptile")
                else:
                    # tail PVs rotate through the (by then idle) scores
                    # banks so the last four PV tiles land in four distinct
                    # banks and never wait on a normalize read
                    ptile = pool.tile([P, BCOLS], F32, tag="scores",
                                      name="tailpv")[:, :D + 1]
                state[cc] = ptile
                return ptile, t

            def mm(cc, j):
                def run():
                    if j == 0:
                        ptile, t = start_tile(cc)
                    else:
                        ptile, t = state[cc], 2 * g + cc
                    c0 = colof[(g, j)] + (0 if (cc == 1 and j == t) else cc * P)
                    nc.tensor.matmul(
                        ptile[:],
                        lhsT=etile[:, c0:c0 + P],
                        rhs=vtile[:, j, :],
                        start=(j == 0), stop=(j == t),
                    )
                    if j == t:
                        rec = rpool.tile([P, 1], F32)
                        nc.vector.reciprocal(rec[:], ptile[:, D:D + 1])
                        if act_norm and cc == 1:
                            # tail tiles: run half the normalizes on the (by
                            # then idle) ACT engine so the final normalize ->
                            # store chain isn't serialized on DVE
                            nc.scalar.activation(
                                osb[:, t, :], ptile[:, 0:D],
                                mybir.ActivationFunctionType.Copy,
                                scale=rec[:])
                        else:
                            nc.vector.tensor_scalar_mul(osb[:, t, :],
                                                        ptile[:, 0:D], rec[:])
                        # store: list fires after cc==1; dict fires per-cc
                        if isinstance(store, dict):
                            for eng, dst, src in store.get(cc, ()):
                                eng.dma_start(dst, src)
                        elif cc == 1 and store is not None:
                            for eng, dst, src in store:
                                eng.dma_start(dst, src)
                return run

            for cc in range(2):
                for j in range(2 * g + cc + 1):
                    pvwork.append(mm(cc, j))

        def drain_pv(n):
            for _ in range(min(n, len(pvwork))):
                pvwork.pop(0)()

        for u in range(NU):
            ktile = kpool.tile([P, S], QKDT)
            if u == 0:
                # The cost model serializes DMA transfers in ready-order, so
                # split k into 3 pieces on SP (consumption order) and put the
                # tiny opening-q pieces + masks on the Pool SWDGE queue; the
                # ACT queue stays clear for exp dispatch (its LoadActFuncSet
                # blocks early DMA generation there).
                nc.sync.dma_start(ktile[:, :384], kT[u][:, :384])
                nc.sync.dma_start(ktile[:, 384:1152], kT[u][:, 384:1152])
                nc.sync.dma_start(ktile[:, 1152:], kT[u][:, 1152:])
            else:
                nc.sync.dma_start(ktile[:], kT[u])
            vtile = vpool.tile([P, NT, D + 1], BF16)
            for hl in range(NHL):
                qtile = qpool.tile([P, S], QKDT)
                if u == 0 and hl == 0:
                    # groups run big->small and the first batch is 3 chunks:
                    # the opening batches only read q columns [1792:2048]
                    nc.gpsimd.dma_start(qtile[:, 1792:], qT[u, hl][:, 1792:])
                    nc.gpsimd.dma_start(mtile[:], masks[:])
                    nc.gpsimd.dma_start(qtile[:, 1536:1792],
                                        qT[u, hl][:, 1536:1792])
                    nc.sync.dma_start(vtile[:], vp[u])
                    nc.sync.dma_start(qtile[:, :1536], qT[u, hl][:, :1536])
                elif hl == 0:
                    nc.sync.dma_start(qtile[:], qT[u, hl])
                    nc.sync.dma_start(vtile[:], vp[u])
                else:
                    nc.sync.dma_start(qtile[:], qT[u, hl])
                osb = opool.tile([P, NT, D], F32)
                etile = epool.tile([P, HEADCOLS], BF16)
                last_head = (u == NU - 1 and hl == NHL - 1)
                # groups big->small: every head ends on tiny PV work and the
                # next head opens with big QK batches, keeping ScalarE fed
                # across head boundaries (and the kernel tail short)
                gs = list(range(NG - 1, -1, -1))

                # chunk stream for this head: full 256-wide chunks of a PAIR
                # of groups, then their two 128-wide diagonal half-chunks
                # back-to-back. Keeps every 256-wide PSUM write 256-aligned so
                # no matmul output crosses a PSUM bank boundary.
                chunks = []
                for ga, gb in zip(gs[0::2], gs[1::2]):
                    for j in range(2 * ga + 1):
                        chunks.append((ga, j, SQG, ga * SQG))
                    chunks.append((ga, 2 * ga + 1, P, ga * SQG + P))
                    chunks.append((gb, 2 * gb + 1, P, gb * SQG + P))
                    for j in range(2 * gb + 1):
                        chunks.append((gb, j, SQG, gb * SQG))
                remaining = {g: 2 * g + 2 for g in gs}
                colof = {}
                acc = 0
                for (g, j, w, qc) in chunks:
                    colof[(g, j)] = acc
                    acc += w

                def do_batch(batch, bcols):
                    stile = spsum.tile([P, BCOLS], F32, tag="scores")
                    ncols = 0
                    for (g, j, w, qc) in batch:
                        # diagonal chunks get the causal mask folded in on PE:
                        # a second matmul accumulates -1e9 into the masked
                        # triangle (exp then gives exactly 0), so no DVE mask
                        # multiply is ever needed downstream
                        diag = (j == 2 * g or j == 2 * g + 1)
                        nc.tensor.matmul(
                            stile[:, ncols:ncols + w],
                            lhsT=ktile[:, j * P:(j + 1) * P],
                            rhs=qtile[:, qc:qc + w],
                            start=True, stop=not diag,
                        )
                        if diag:
                            nc.tensor.matmul(
                                stile[:, ncols:ncols + P],
                                lhsT=mtile[:, 0, :],
                                rhs=mtile[:, 1, :],
                                start=False, stop=True,
                            )
                        ncols += w
                    e0 = colof[(batch[0][0], batch[0][1])]
                    nc.scalar.activation(
                        etile[:, e0:e0 + ncols],
                        stile[:, :ncols],
                        mybir.ActivationFunctionType.Exp,
                        scale=SCALE,
                    )
                    # a group is complete once all its chunks are exp'd.
                    # Groups complete big->small (descending) which keeps the
                    # steady-state pipeline tight; only the LAST head's final
                    # batch (groups 1+0 complete together) queues ascending,
                    # so g0's normalize+store starts ~1us earlier at the tail.
                    for (g, j, w, qc) in batch:
                        remaining[g] -= 1
                    done = [g for g in dict.fromkeys(c[0] for c in batch)
                            if remaining[g] == 0]
                    if last_head and set(done) == {0, 1}:
                        # only the very last batch flips to ascending: stores
                        # are per-group there, so g0's chain can lead. Other
                        # multi-group batches must stay descending -- e.g.
                        # g2's store reads g3's osb tiles and therefore has
                        # to queue after g3's normalizes.
                        done = sorted(done)
                    for g in done:
                        if True:
                            if last_head and g == NG // 2:
                                # tiles 8..15 done early under big->small
                                # order: store them as soon as ready
                                store = [(nc.gpsimd, out[u, hl][:, NG:, :],
                                          osb[:, NG:, :])]
                            elif last_head and g == 2:
                                store = [(nc.sync, out[u, hl][:, 4:6, :],
                                          osb[:, 4:6, :]),
                                         (nc.gpsimd, out[u, hl][:, 6:NG, :],
                                          osb[:, 6:NG, :])]
                            elif last_head and g == 1:
                                # g1 is queued after g0 (ascending), so this
                                # is the final store of the kernel
                                store = [(nc.sync, out[u, hl][:, 2:4, :],
                                          osb[:, 2:4, :])]
                            elif last_head and g == 0:
                                store = [(nc.scalar, out[u, hl][:, 0:2, :],
                                          osb[:, 0:2, :])]
                            elif g == gs[-1]:
                                # descending order: g0 is queued last, so the
                                # whole-head store (which reads every osb
                                # tile) rides on its final normalize
                                store = [(nc.gpsimd, out[u, hl], osb[:])]
                            else:
                                store = None
                            # the last head's final two PV groups run after
                            # the last exp: pull their PSUM from the (by then
                            # idle) scores pool so they don't serialize on the
                            # two pv banks behind group 2's normalize.
                            pvpool = spsum if (last_head and g <= 1) else None
                            queue_pv(vtile, etile, colof, osb, g, store,
                                     pvpool,
                                     act_norm=(last_head and g <= 3))

                batch, bcols = [], 0
                nbatch = 0
                for ch in chunks:
                    # the first batch of each head is kept small (3 chunks) so
                    # the next head's opening QK+exp slots in right behind the
                    # previous head's final exp instead of stalling ScalarE.
                    # On the last head the final batch is split at the g1/g0
                    # boundary so group 1 completes (and its PV+normalize
                    # chain starts) while group 0's exp still runs.
                    cap = ((512 if u == 0 and hl == 0 else 1024)
                           if nbatch == 0 else BCOLS)
                    split = (last_head and ch[0] == 0 and batch
                             and batch[-1][0] == 1)
                    if bcols + ch[2] > cap or split:
                        do_batch(batch, bcols)
                        nbatch += 1
                        batch, bcols = [], 0
                        # on the last head drain harder so only the final two
                        # groups' PV work remains after the last exp -- but
                        # defer entirely on the last two boundaries so the
                        # tail masks lead the in-order DVE stream
                        if last_head and nbatch >= 11:
                            pass
                        else:
                            drain_pv(KPV + (14 if last_head else 0))
                    batch.append(ch)
                    bcols += ch[2]
                if batch:
                    do_batch(batch, bcols)
        drain_pv(len(pvwork))

    nc.compile()
    return nc


def _prep_inputs(q, k, v):
    """Host-side sharding/layout. Returns in_maps for the 8 cores."""
    q = np.ascontiguousarray(np.asarray(q, dtype=np.float32))
    k = np.ascontiguousarray(np.asarray(k, dtype=np.float32))
    v = np.ascontiguousarray(np.asarray(v, dtype=np.float32))

    # [B, H(K), D, S] transposed views, contiguous
    qt_all = np.ascontiguousarray(q.reshape(B, S, H, D).transpose(0, 2, 3, 1))
    kt_all = np.ascontiguousarray(k.reshape(B, S, HK, D).transpose(0, 2, 3, 1))
    v4 = v.reshape(B, S, HK, D)

    # PE-side causal mask for scoresT diagonal blocks: masks[:,0,:] is
    # (-1e9 * [sk > sq])^T, masks[:,1,:] the identity; lhsT.T @ rhs adds
    # -1e9 to masked positions (exp -> 0) and exactly 0 elsewhere.
    r = np.arange(P)[:, None]
    c = np.arange(P)[None, :]
    masks = np.empty((P, 2, P), dtype=ml_dtypes.bfloat16)
    masks[:, 0, :] = np.where(c > r, -1e9, 0.0).astype(ml_dtypes.bfloat16)
    masks[:, 1, :] = np.eye(P, dtype=np.float32)

    in_maps = []
    for core in range(NCORES):
        qs = np.empty((NU, NHL, P, S), dtype=ml_dtypes.bfloat16)
        ks = np.empty((NU, P, S), dtype=ml_dtypes.bfloat16)
        vs = np.empty((NU, P, NT, D + 1), dtype=ml_dtypes.bfloat16)
        for ui in range(NU):
            ug = core * NU + ui
            b, kv = divmod(ug, HK)
            qs[ui] = qt_all[b, kv * REP:(kv + 1) * REP]
            ks[ui] = kt_all[b, kv]
            # V' chunks: [sk_in_chunk, chunk, d] with ones in column D
            vu = v4[b, :, kv, :].reshape(NT, P, D).transpose(1, 0, 2)
            vs[ui, :, :, :D] = vu.astype(ml_dtypes.bfloat16)
            vs[ui, :, :, D] = 1.0
        in_maps.append({"qT": qs, "kT": ks, "vp": vs, "masks": masks})
    return in_maps


def _assemble(results):
    res = np.empty((B, S, H, D), dtype=np.float32)
    for core in range(NCORES):
        arr = results[core]["out"]  # [NU, NHL, P(sq), NT, D]
        for ui in range(NU):
            ug = core * NU + ui
            b, kv = divmod(ug, HK)
            # [NHL, P, NT, D] -> [(NT P) = S, NHL, D]
            blk = arr[ui].transpose(2, 1, 0, 3).reshape(S, NHL, D)
            res[b, :, kv * REP:(kv + 1) * REP, :] = blk
    return res.reshape(B * S, H * D)


def kernel(q, k, v, seq_lens=None, **_unused):
    key = "prog"
    if key not in _CACHE:
        _CACHE[key] = _build_program()
    nc = _CACHE[key]
    in_maps = _prep_inputs(q, k, v)
    res = run_bass_kernel_spmd(nc, in_maps, list(range(NCORES)))
    return _assemble(res.results)

